# revision 1
# baseline (speedup 1.0000x reference)
"""Trainium2 Bass kernel for nn_Attention_Layer (B=4, S=2048, D=1024, fp32).

Sharding: 8 cores = 4 batches x 2 query-halves. Each core computes K/V for
its whole batch (from x^T, pre-transposed on host) and attention for its
1024-query half. Scores are built transposed ([k, q] layout) so the softmax
denominator folds into a per-partition scalar at the output, and the
attn @ V contraction needs no on-device transpose of the attention matrix.

Compute dtypes: projections and scores run the PE in float32r (full-rate
fp32 path); exp runs on ACT in fp32; the attention-weights @ V product runs
in bf16 (weights are probabilities, V rounding averages out).

Weights are host-relaid to [EO, P, DO*P] so each e-tile weight load is one
fully contiguous per-partition DMA (strided 512B-chunk loads starve the PE).
A short burst of dummy matmuls at kernel start keeps the PE's HAM clock
gate warm through the initial DMA wait.
"""

import numpy as np

import concourse.mybir as mybir
import concourse.tile as tile
from concourse import bacc
from concourse.bass_utils import run_bass_kernel_spmd

B, S, D = 4, 2048, 1024
P = 128
HALF = S // 2            # queries per core; also the k-half processed per phase
EO = D // P              # 8 e-tiles (feature dim outer)
DO = D // P              # 8 d-tiles (contraction outer)
KO = S // P              # 16 k-tiles (global)
QT = HALF // P           # 8 q-tiles per core
SCALE = 1.0 / np.sqrt(D)

F32 = mybir.dt.float32
F32R = mybir.dt.float32r
BF16 = mybir.dt.bfloat16


def build_nc():
    nc = bacc.Bacc("TRN2", target_bir_lowering=False)

    xT = nc.dram_tensor("xT", [D, S], F32R, kind="ExternalInput")
    Wk = nc.dram_tensor("Wk", [EO, P, DO * P], F32R, kind="ExternalInput")
    Wq = nc.dram_tensor("Wq", [EO, P, DO * P], F32R, kind="ExternalInput")
    Wv = nc.dram_tensor("Wv", [D, D], F32R, kind="ExternalInput")
    bkT = nc.dram_tensor("bkT", [P, EO], F32, kind="ExternalInput")
    bqT = nc.dram_tensor("bqT", [P, EO], F32, kind="ExternalInput")
    bv = nc.dram_tensor("bv", [P, D], BF16, kind="ExternalInput")
    y = nc.dram_tensor("y", [HALF, D], F32, kind="ExternalOutput")

    xTr = xT.ap().rearrange("(do p) s -> p do s", p=P)
    Wvr = Wv.ap().rearrange("(do p) e -> p do e", p=P)

    with tile.TileContext(nc) as tc:
        with (
            tc.tile_pool(name="xts", bufs=2) as xts_pool,       # 32KB
            tc.tile_pool(name="wke", bufs=2) as wke_pool,
            tc.tile_pool(name="wve", bufs=1) as wve_pool,       # 32KB
            tc.tile_pool(name="kt", bufs=1) as k_pool,          # 32KB
            tc.tile_pool(name="qt", bufs=1) as q_pool,          # 32KB
            tc.tile_pool(name="vt", bufs=1) as v_pool,          # 32KB
            tc.tile_pool(name="pt", bufs=1) as p_pool,          # 32KB
            tc.tile_pool(name="outp", bufs=2) as out_pool,      # 4KB
            tc.tile_pool(name="small", bufs=1) as small_pool,
            tc.tile_pool(name="ps", bufs=6, space="PSUM") as ps_pool,
            tc.tile_pool(name="avz", bufs=1, space="PSUM") as avz_pool,
        ):
            bk_sb = small_pool.tile([P, EO], F32, tag="bk")
            bq_sb = small_pool.tile([P, EO], F32, tag="bq")
            bv_sb = small_pool.tile([P, D], BF16, tag="bv")
            ones_sb = small_pool.tile([P, 1], BF16, tag="ones")
            rz_sb = small_pool.tile([P, QT], F32, tag="rz")
            nc.vector.memset(ones_sb[:], 1.0)

            def emit_deferred_small_loads():
                nc.sync.dma_start(bk_sb[:], bkT[:, :])
                nc.sync.dma_start(bq_sb[:], bqT[:, :])
                nc.sync.dma_start(bv_sb[:], bv[:, :])

            # keep the PE busy (HAM warm) while the first x/W DMAs land
            warm_ps = avz_pool.tile([1, 8], F32, tag="warm")
            for _ in range(120):
                nc.tensor.matmul(
                    warm_ps[:, 0:1], ones_sb[:], ones_sb[:],
                    start=True, stop=True,
                )

            q_sb = q_pool.tile([P, EO, HALF], F32R, tag="qt")
            v_sb = v_pool.tile([P, KO, D], BF16, tag="vt")
            p_sb = p_pool.tile([P, KO, D], BF16, tag="pt")

            # ---- projections + scores, one k-half at a time --------------
            # query half is always s in [0, HALF) after the host swap.
            for kh in range(2):
                k_sb = k_pool.tile([P, EO, HALF], F32R, tag="kt")
                xq = []
                for sq in range(2):
                    t = xts_pool.tile([P, DO, 512], F32R, tag="xts")
                    for do in range(DO):   # split DMA across queues
                        nc.sync.dma_start(
                            t[:, do, :],
                            xTr[:, do, kh * HALF + sq * 512 : kh * HALF + sq * 512 + 512],
                        )
                    xq.append(t)
                if kh == 0:
                    emit_deferred_small_loads()
                # K^T (and Q^T in the query half): one contiguous wke load
                # per e-tile feeding both 512-wide s-quarters. The very first
                # e-tile runs its quarter-0 group before quarter 1 arrives.
                projs = [(Wk, bk_sb, k_sb)]
                if kh == 0:
                    projs.append((Wq, bq_sb, q_sb))
                for pi, (Wt, b_sb, dst_sb) in enumerate(projs):
                    for eo in range(EO):
                        wke = wke_pool.tile([P, DO, P], F32R, tag="wke")
                        nc.sync.dma_start(
                            wke[:], Wt[eo].unsqueeze(0).rearrange(
                                "o p (do e) -> (o p) do e", do=DO
                            ),
                        )
                        split_first = kh == 0 and pi == 0 and eo == 0
                        for sq in range(2):
                            ps0 = ps_pool.tile([P, 512], F32, tag="ps")
                            for do in range(DO):
                                nc.tensor.matmul(
                                    ps0[:], wke[:, do, :], xq[sq][:, do, :],
                                    start=(do == 0), stop=(do == DO - 1),
                                )
                            nc.vector.tensor_scalar_add(
                                dst_sb[:, eo, sq * 512 : sq * 512 + 512],
                                ps0[:],
                                b_sb[:, eo : eo + 1],
                            )
                        del split_first
                # V[s-half, :]: one ldweights per (ktl, do) serving both
                # 512-wide e-blocks; Wv d-slices resident for the half.
                wve = wve_pool.tile([P, DO, D], F32R, tag="wve")
                for do in range(DO):
                    nc.sync.dma_start(wve[:, do, :], Wvr[:, do, :])
                for ktl in range(8):
                    ko = kh * 8 + ktl
                    ps0 = ps_pool.tile([P, 512], F32, tag="ps")
                    ps1 = ps_pool.tile([P, 512], F32, tag="ps")
                    for do in range(DO):
                        xkt = xq[ktl // 4][:, do, (ktl % 4) * P : (ktl % 4) * P + P]
                        nc.tensor.matmul(
                            ps0[:], xkt, wve[:, do, 0:512],
                            start=(do == 0), stop=(do == DO - 1),
                        )
                        nc.tensor.matmul(
                            ps1[:], xkt, wve[:, do, 512:1024],
                            start=(do == 0), stop=(do == DO - 1),
                        )
                    nc.vector.tensor_tensor(
                        v_sb[:, ko, 0:512], ps0[:], bv_sb[:, 0:512],
                        mybir.AluOpType.add,
                    )
                    nc.vector.tensor_tensor(
                        v_sb[:, ko, 512:1024], ps1[:], bv_sb[:, 512:1024],
                        mybir.AluOpType.add,
                    )
                # scores^T for this k-half: exp(K^T.T @ Q^T / sqrt(D));
                # one ldweights per (ktl, eo) serving both q-blocks.
                for ktl in range(8):
                    ko = kh * 8 + ktl
                    ps0 = ps_pool.tile([P, 512], F32, tag="ps")
                    ps1 = ps_pool.tile([P, 512], F32, tag="ps")
                    for eo in range(EO):
                        kt_ap = k_sb[:, eo, ktl * P : (ktl + 1) * P]
                        nc.tensor.matmul(
                            ps0[:], kt_ap, q_sb[:, eo, 0:512],
                            start=(eo == 0), stop=(eo == EO - 1),
                        )
                        nc.tensor.matmul(
                            ps1[:], kt_ap, q_sb[:, eo, 512:1024],
                            start=(eo == 0), stop=(eo == EO - 1),
                        )
                    nc.scalar.activation(
                        p_sb[:, ko, 0:512], ps0[:],
                        mybir.ActivationFunctionType.Exp, scale=float(SCALE),
                    )
                    nc.scalar.activation(
                        p_sb[:, ko, 512:1024], ps1[:],
                        mybir.ActivationFunctionType.Exp, scale=float(SCALE),
                    )

            # ---- attention output: (P^T.T @ V) * (1/Z) -------------------
            zt = avz_pool.tile([P, QT], F32, tag="avz")
            for qt in range(QT):
                av0 = ps_pool.tile([P, 512], F32, tag="ps")
                av1 = ps_pool.tile([P, 512], F32, tag="ps")
                for ko in range(KO):
                    lhs = p_sb[:, ko, qt * P : (qt + 1) * P]
                    nc.tensor.matmul(
                        av0[:], lhs, v_sb[:, ko, 0:512],
                        start=(ko == 0), stop=(ko == KO - 1),
                    )
                    nc.tensor.matmul(
                        av1[:], lhs, v_sb[:, ko, 512:1024],
                        start=(ko == 0), stop=(ko == KO - 1),
                    )
                    nc.tensor.matmul(
                        zt[:, qt : qt + 1], lhs, ones_sb[:],
                        start=(ko == 0), stop=(ko == KO - 1),
                    )
                nc.vector.reciprocal(rz_sb[:, qt : qt + 1], zt[:, qt : qt + 1])
                o0 = out_pool.tile([P, 512], F32, tag="outp")
                o1 = out_pool.tile([P, 512], F32, tag="outp")
                nc.vector.tensor_scalar_mul(o0[:], av0[:], rz_sb[:, qt : qt + 1])
                nc.vector.tensor_scalar_mul(o1[:], av1[:], rz_sb[:, qt : qt + 1])
                nc.sync.dma_start(y[qt * P : (qt + 1) * P, 0:512], o0[:])
                nc.sync.dma_start(y[qt * P : (qt + 1) * P, 512:1024], o1[:])

    nc.finalize()
    return nc


_NC_CACHE = None


def make_in_maps(x, Wk, bk, Wq, bq, Wv, bv):
    import ml_dtypes

    x = np.asarray(x, dtype=np.float32)
    def _wre(W):
        # [D, D] -> [EO, P(part), DO*P] so each e-tile slice is one
        # fully contiguous per-partition DMA
        W = np.asarray(W, np.float32).reshape(DO, P, EO, P)
        return np.ascontiguousarray(
            W.transpose(2, 1, 0, 3).reshape(EO, P, DO * P)
        )

    Wk = _wre(Wk)
    Wq = _wre(Wq)
    Wv = np.ascontiguousarray(np.asarray(Wv, np.float32))
    bkT = np.ascontiguousarray(np.asarray(bk, np.float32).reshape(EO, P).T)
    bqT = np.ascontiguousarray(np.asarray(bq, np.float32).reshape(EO, P).T)
    bv2 = np.ascontiguousarray(
        np.broadcast_to(
            np.asarray(bv, np.float32).reshape(1, D), (P, D)
        ).astype(ml_dtypes.bfloat16)
    )

    in_maps = []
    for c in range(8):
        b, h = c // 2, c % 2
        xTb = np.ascontiguousarray(x[b].T)          # [D, S]
        if h == 1:
            # swap the s-halves so this core's query half is always first
            xTb = np.ascontiguousarray(
                np.concatenate([xTb[:, HALF:], xTb[:, :HALF]], axis=1)
            )
        in_maps.append(
            {
                "xT": xTb,
                "Wk": Wk, "Wq": Wq, "Wv": Wv,
                "bkT": bkT, "bqT": bqT, "bv": bv2,
            }
        )
    return in_maps


def gather_out(results):
    out = np.empty((B, S, D), dtype=np.float32)
    for c in range(8):
        b, h = c // 2, c % 2
        out[b, h * HALF : (h + 1) * HALF, :] = results[c]["y"]
    return out


def kernel(x, Wk, bk, Wq, bq, Wv, bv):
    global _NC_CACHE
    if _NC_CACHE is None:
        _NC_CACHE = build_nc()
    in_maps = make_in_maps(x, Wk, bk, Wq, bq, Wv, bv)
    res = run_bass_kernel_spmd(_NC_CACHE, in_maps, list(range(8)))
    return gather_out(res.results)



# revision 2
# speedup vs baseline: 1.3177x; 1.3177x over previous
"""Trainium2 Bass kernel for nn_Attention_Layer (B=4, S=2048, D=1024, fp32).

Sharding: 8 cores = 4 batches x 2 KEY-halves (flash-attention style).
Each core computes K/V projections for its 1024-key half only, Q for all
2048 queries, scores/softmax-numerator against its key half, and ships the
UNNORMALIZED attention output N = exp(S)@V plus the per-query partial
denominator Z. The host combines halves: y = (N0 + N1) / (Z0 + Z1).
vs. the query-split layout this removes the duplicated K/V projection
(each projection is computed exactly twice across the pair either way,
but here the duplicated one is Q only: 2.1 GFLOP less per core).

Compute dtypes: projections run the PE in bf16 (x and W shipped bf16);
the scores matmul runs in fp8-e4m3 with MatmulPerfMode.DoubleRow (two
128-deep k-tiles contracted per instruction at 0.5 cycles/row); exp on
ACT in fp32; attn-weights @ V in bf16. The K-projection bias is dropped:
score terms that depend only on the query row cancel in softmax, so
softmax_k((q+bq)·(k+bk)) == softmax_k((q+bq)·k).

Scores are built transposed ([k, q]) so the attn @ V contraction needs no
on-device transpose; Z rides along as a 1-wide matmul against ones.
Weights are host-relaid to [EO, P, DO*P] so each e-tile weight load is one
fully contiguous per-partition DMA. A short burst of dummy matmuls at
kernel start keeps the PE's HAM clock gate warm through the initial DMA
wait.
"""

import numpy as np

import concourse.mybir as mybir
import concourse.tile as tile
from concourse import bacc
from concourse.bass_utils import run_bass_kernel_spmd

B, S, D = 4, 2048, 1024
P = 128
KH = S // 2              # keys per core
EO = D // P              # 8 e-tiles (feature dim outer)
DO = D // P              # 8 d-tiles (contraction outer)
KO = KH // P             # 8 key 128-tiles per core
QC = S // 512            # 4 query 512-chunks
KC = KH // 512           # 2 key 512-chunks
EP = EO // 2             # 4 eo-pairs for DoubleRow
QT = S // P              # 16 query 128-tiles
SCALE = 1.0 / np.sqrt(D)

F32 = mybir.dt.float32
BF16 = mybir.dt.bfloat16
FP8 = mybir.dt.float8e4
DR = mybir.MatmulPerfMode.DoubleRow


def build_nc():
    nc = bacc.Bacc("TRN2", target_bir_lowering=False)

    xT = nc.dram_tensor("xT", [D, S], BF16, kind="ExternalInput")
    Wk = nc.dram_tensor("Wk", [EO, P, DO * P], BF16, kind="ExternalInput")
    Wq = nc.dram_tensor("Wq", [EO, P, DO * P], BF16, kind="ExternalInput")
    Wv = nc.dram_tensor("Wv", [D, D], BF16, kind="ExternalInput")
    bqT = nc.dram_tensor("bqT", [P, EO], F32, kind="ExternalInput")
    bv = nc.dram_tensor("bv", [P, D], BF16, kind="ExternalInput")
    y = nc.dram_tensor("y", [S, D], BF16, kind="ExternalOutput")
    z = nc.dram_tensor("z", [P, QT], F32, kind="ExternalOutput")

    xTr = xT.ap().rearrange("(do p) s -> p do s", p=P)
    Wvr = Wv.ap().rearrange("(do p) e -> p do e", p=P)

    with tile.TileContext(nc) as tc:
        with (
            tc.tile_pool(name="xts", bufs=1) as xts_pool,       # 32KB
            tc.tile_pool(name="wke", bufs=2) as wke_pool,       # 2x2KB
            tc.tile_pool(name="wve", bufs=1) as wve_pool,       # 16KB
            tc.tile_pool(name="k8", bufs=1) as k8_pool,         # 8KB
            tc.tile_pool(name="q8", bufs=1) as q8_pool,         # 16KB
            tc.tile_pool(name="vt", bufs=1) as v_pool,          # 16KB
            tc.tile_pool(name="pt", bufs=1) as p_pool,          # 32KB
            tc.tile_pool(name="outp", bufs=4) as out_pool,      # 4x1KB
            tc.tile_pool(name="small", bufs=1) as small_pool,
            tc.tile_pool(name="ps", bufs=6, space="PSUM") as ps_pool,
            tc.tile_pool(name="avz", bufs=1, space="PSUM") as avz_pool,
        ):
            bq_sb = small_pool.tile([P, EO], F32, tag="bq")
            bv_sb = small_pool.tile([P, D], BF16, tag="bv")
            ones_sb = small_pool.tile([P, 1], BF16, tag="ones")
            zs_sb = small_pool.tile([P, QT], F32, tag="zs")
            nc.vector.memset(ones_sb[:], 1.0)

            # keep the PE busy (HAM warm) while the first x/W DMAs land
            warm_ps = avz_pool.tile([1, 8], F32, tag="warm")
            for _ in range(120):
                nc.tensor.matmul(
                    warm_ps[:, 0:1], ones_sb[:], ones_sb[:],
                    start=True, stop=True,
                )

            xts = xts_pool.tile([P, DO, S], BF16, tag="xts")
            k8 = k8_pool.tile([P, EO, KH], FP8, tag="k8")
            q8 = q8_pool.tile([P, EO, S], FP8, tag="q8")
            v_sb = v_pool.tile([P, KO, D], BF16, tag="vt")
            p_sb = p_pool.tile([P, KO, S], BF16, tag="pt")

            # x DMAs: key chunks first, split per do across queues
            for sq in range(2):
                for do in range(DO):
                    nc.sync.dma_start(
                        xts[:, do, sq * 512 : sq * 512 + 512],
                        xTr[:, do, sq * 512 : sq * 512 + 512],
                    )
            nc.sync.dma_start(bq_sb[:], bqT[:, :])
            nc.sync.dma_start(bv_sb[:], bv[:, :])
            for sq in range(2, QC):
                for do in range(DO):
                    nc.sync.dma_start(
                        xts[:, do, sq * 512 : sq * 512 + 512],
                        xTr[:, do, sq * 512 : sq * 512 + 512],
                    )

            # ---- K projection (key half, no bias: it cancels in softmax) --
            for eo in range(EO):
                wke = wke_pool.tile([P, DO, P], BF16, tag="wke")
                nc.sync.dma_start(
                    wke[:], Wk[eo].unsqueeze(0).rearrange(
                        "o p (do e) -> (o p) do e", do=DO
                    ),
                )
                for kc in range(KC):
                    ps0 = ps_pool.tile([P, 512], F32, tag="ps")
                    for do in range(DO):
                        nc.tensor.matmul(
                            ps0[:], wke[:, do, :],
                            xts[:, do, kc * 512 : kc * 512 + 512],
                            start=(do == 0), stop=(do == DO - 1),
                        )
                    nc.scalar.activation(
                        k8[:, eo, kc * 512 : kc * 512 + 512], ps0[:],
                        mybir.ActivationFunctionType.Copy,
                    )

            # ---- Q projection (all queries, bias via ACT Identity) -------
            for eo in range(EO):
                wqe = wke_pool.tile([P, DO, P], BF16, tag="wke")
                nc.sync.dma_start(
                    wqe[:], Wq[eo].unsqueeze(0).rearrange(
                        "o p (do e) -> (o p) do e", do=DO
                    ),
                )
                for qc in range(QC):
                    ps0 = ps_pool.tile([P, 512], F32, tag="ps")
                    for do in range(DO):
                        nc.tensor.matmul(
                            ps0[:], wqe[:, do, :],
                            xts[:, do, qc * 512 : qc * 512 + 512],
                            start=(do == 0), stop=(do == DO - 1),
                        )
                    nc.scalar.activation(
                        q8[:, eo, qc * 512 : qc * 512 + 512], ps0[:],
                        mybir.ActivationFunctionType.Identity,
                        bias=bq_sb[:, eo : eo + 1],
                    )

            # V weights can stream in during the scores phase
            wve = wve_pool.tile([P, DO, D], BF16, tag="wve")
            for do in range(DO):
                nc.sync.dma_start(wve[:, do, :], Wvr[:, do, :])

            # ---- scores^T: exp(K.T @ Q / sqrt(D)), fp8 DoubleRow ---------
            for qc in range(QC):
                for kt in range(KO):
                    ps0 = ps_pool.tile([P, 512], F32, tag="ps")
                    for ep in range(EP):
                        nc.tensor.matmul(
                            ps0[:],
                            k8[:, 2 * ep : 2 * ep + 2, kt * P : (kt + 1) * P],
                            q8[:, 2 * ep : 2 * ep + 2, qc * 512 : qc * 512 + 512],
                            start=(ep == 0), stop=(ep == EP - 1),
                            perf_mode=DR,
                        )
                    nc.scalar.activation(
                        p_sb[:, kt, qc * 512 : qc * 512 + 512], ps0[:],
                        mybir.ActivationFunctionType.Exp, scale=float(SCALE),
                    )

            # ---- V projection (key half) ---------------------------------
            for kt in range(KO):
                ps0 = ps_pool.tile([P, 512], F32, tag="ps")
                ps1 = ps_pool.tile([P, 512], F32, tag="ps")
                for do in range(DO):
                    xkt = xts[:, do, kt * P : (kt + 1) * P]
                    nc.tensor.matmul(
                        ps0[:], xkt, wve[:, do, 0:512],
                        start=(do == 0), stop=(do == DO - 1),
                    )
                    nc.tensor.matmul(
                        ps1[:], xkt, wve[:, do, 512:1024],
                        start=(do == 0), stop=(do == DO - 1),
                    )
                nc.vector.tensor_tensor(
                    v_sb[:, kt, 0:512], ps0[:], bv_sb[:, 0:512],
                    mybir.AluOpType.add,
                )
                nc.vector.tensor_tensor(
                    v_sb[:, kt, 512:1024], ps1[:], bv_sb[:, 512:1024],
                    mybir.AluOpType.add,
                )

            # ---- attention numerator N = P^T.T @ V and denominator Z -----
            zt = avz_pool.tile([P, QT], F32, tag="avz")
            for qt in range(QT):
                av0 = ps_pool.tile([P, 512], F32, tag="ps")
                av1 = ps_pool.tile([P, 512], F32, tag="ps")
                for ko in range(KO):
                    lhs = p_sb[:, ko, qt * P : (qt + 1) * P]
                    nc.tensor.matmul(
                        av0[:], lhs, v_sb[:, ko, 0:512],
                        start=(ko == 0), stop=(ko == KO - 1),
                    )
                    nc.tensor.matmul(
                        av1[:], lhs, v_sb[:, ko, 512:1024],
                        start=(ko == 0), stop=(ko == KO - 1),
                    )
                    nc.tensor.matmul(
                        zt[:, qt : qt + 1], lhs, ones_sb[:],
                        start=(ko == 0), stop=(ko == KO - 1),
                    )
                o0 = out_pool.tile([P, 512], BF16, tag="outp")
                o1 = out_pool.tile([P, 512], BF16, tag="outp")
                nc.scalar.activation(
                    o0[:], av0[:], mybir.ActivationFunctionType.Copy
                )
                nc.scalar.activation(
                    o1[:], av1[:], mybir.ActivationFunctionType.Copy
                )
                nc.sync.dma_start(y[qt * P : (qt + 1) * P, 0:512], o0[:])
                nc.sync.dma_start(y[qt * P : (qt + 1) * P, 512:1024], o1[:])
            nc.scalar.activation(
                zs_sb[:], zt[:], mybir.ActivationFunctionType.Copy
            )
            nc.sync.dma_start(z[:, :], zs_sb[:])

    nc.finalize()
    return nc


_NC_CACHE = None


def make_in_maps(x, Wk, bk, Wq, bq, Wv, bv):
    import ml_dtypes

    bf16 = ml_dtypes.bfloat16
    x = np.asarray(x, dtype=np.float32)

    def _wre(W):
        # [D, D] -> [EO, P(part), DO*P] so each e-tile slice is one
        # fully contiguous per-partition DMA
        W = np.asarray(W, np.float32).reshape(DO, P, EO, P)
        return np.ascontiguousarray(
            W.transpose(2, 1, 0, 3).reshape(EO, P, DO * P).astype(bf16)
        )

    Wk8 = _wre(Wk)
    Wq8 = _wre(Wq)
    Wv8 = np.ascontiguousarray(np.asarray(Wv, np.float32).astype(bf16))
    bqT = np.ascontiguousarray(np.asarray(bq, np.float32).reshape(EO, P).T)
    bv2 = np.ascontiguousarray(
        np.broadcast_to(
            np.asarray(bv, np.float32).reshape(1, D), (P, D)
        ).astype(bf16)
    )

    in_maps = []
    for c in range(8):
        b, kh = c // 2, c % 2
        xTb = np.ascontiguousarray(x[b].T.astype(bf16))    # [D, S]
        if kh == 1:
            # swap the s-halves so this core's key half is always cols [0, KH)
            xTb = np.ascontiguousarray(
                np.concatenate([xTb[:, KH:], xTb[:, :KH]], axis=1)
            )
        in_maps.append(
            {
                "xT": xTb,
                "Wk": Wk8, "Wq": Wq8, "Wv": Wv8,
                "bqT": bqT, "bv": bv2,
            }
        )
    return in_maps


def gather_out(results):
    out = np.empty((B, S, D), dtype=np.float32)
    for b in range(B):
        r0, r1 = results[2 * b], results[2 * b + 1]
        n0 = np.asarray(r0["y"], dtype=np.float32)          # [S, D]
        n1 = np.asarray(r1["y"], dtype=np.float32)
        z0 = np.asarray(r0["z"], dtype=np.float32).T.reshape(S)
        z1 = np.asarray(r1["z"], dtype=np.float32).T.reshape(S)
        # core 1 sees queries in swapped-half order; map back to global
        n1 = np.concatenate([n1[KH:], n1[:KH]], axis=0)
        z1 = np.concatenate([z1[KH:], z1[:KH]], axis=0)
        out[b] = (n0 + n1) / (z0 + z1)[:, None]
    return out


def kernel(x, Wk, bk, Wq, bq, Wv, bv):
    global _NC_CACHE
    if _NC_CACHE is None:
        _NC_CACHE = build_nc()
    in_maps = make_in_maps(x, Wk, bk, Wq, bq, Wv, bv)
    res = run_bass_kernel_spmd(_NC_CACHE, in_maps, list(range(8)))
    return gather_out(res.results)


# revision 10
# speedup vs baseline: 1.5595x; 1.1835x over previous
"""Trainium2 Bass kernel for nn_Attention_Layer (B=4, S=2048, D=1024, fp32).

Sharding: 8 cores = 4 batches x 2 KEY-halves (flash-attention style).
Each core computes K/V projections for its 1024-key half, Q for a
1024-query half, AllGathers Q across the pair (the gather's rank-order
concat IS the global query order, so every downstream access stays
SPMD-uniform), computes exp-scores against its key half and ships the
UNNORMALIZED attention output N = exp(S)@V plus the per-query partial
denominator Z. The host combines halves: y = (N0 + N1) / (Z0 + Z1).

Compute dtypes: projections run the PE in bf16 (x and W shipped bf16);
the scores matmul runs in fp8-e4m3 with MatmulPerfMode.DoubleRow (two
128-deep k-tiles contracted per instruction, 2x bf16 rate); exp on ACT
in fp32; attn-weights @ V in bf16. The K-projection bias is dropped:
score terms that depend only on the query row cancel in softmax.

Z rides along as column 1024 (all-ones) of the V tile: the attn @ V
matmuls per (qt, ko) are 384+384+257 wide, so every weight (re)load
hides behind a longer moving phase and Z needs no extra 1-wide matmuls.

DMA uses both hardware DGE queues: weights stream on the scalar queue,
x / bounce / outputs on the sync queue, so the first weight tile isn't
stuck behind the x stream. A short burst of dummy matmuls at kernel
start keeps the PE's HAM clock gate warm through the initial DMA wait.
"""

import numpy as np

import concourse.mybir as mybir
import concourse.tile as tile
from concourse import bacc
from concourse.bass_utils import run_bass_kernel_spmd

B, S, D = 4, 2048, 1024
P = 128
KH = S // 2              # keys per core
QH = S // 2              # queries projected per core (CC mode)
EO = D // P              # 8 e-tiles (feature dim outer)
DO = D // P              # 8 d-tiles (contraction outer)
KO = KH // P             # 8 key 128-tiles per core
QC = S // 512            # 4 query 512-chunks
KC = KH // 512           # 2 key 512-chunks
EP = EO // 2             # 4 eo-pairs for DoubleRow
QT = S // P              # 16 query 128-tiles
VW = D + 1               # V free width incl. the ones column for Z
SCALE = 1.0 / np.sqrt(D)

USE_CC = True            # AllGather the Q projection across core pairs

F32 = mybir.dt.float32
BF16 = mybir.dt.bfloat16
FP8 = mybir.dt.float8e4
DR = mybir.MatmulPerfMode.DoubleRow
ACTF = mybir.ActivationFunctionType


def build_nc():
    nc = bacc.Bacc("TRN2", target_bir_lowering=False)

    XW = S if not USE_CC else KH     # x columns actually needed on-core
    xT = nc.dram_tensor("xT", [D, XW], BF16, kind="ExternalInput")
    Wk = nc.dram_tensor("Wk", [EO, P, DO * P], BF16, kind="ExternalInput")
    Wq = nc.dram_tensor("Wq", [EO, P, DO * P], BF16, kind="ExternalInput")
    Wv = nc.dram_tensor("Wv", [D, D], BF16, kind="ExternalInput")
    bqT = nc.dram_tensor("bqT", [P, EO], F32, kind="ExternalInput")
    bv = nc.dram_tensor("bv", [P, D], BF16, kind="ExternalInput")
    y = nc.dram_tensor("y", [S, D], BF16, kind="ExternalOutput")
    z = nc.dram_tensor("z", [P, QT], F32, kind="ExternalOutput")

    xTr = xT.ap().rearrange("(do p) s -> p do s", p=P)
    Wvr = Wv.ap().rearrange("(do p) e -> p do e", p=P)

    with tile.TileContext(nc) as tc:
        with (
            tc.tile_pool(name="xts", bufs=1) as xts_pool,
            tc.tile_pool(name="wk", bufs=1) as wk_pool,         # 16KB
            tc.tile_pool(name="wq", bufs=1) as wq_pool,         # 16KB
            tc.tile_pool(name="wve", bufs=1) as wve_pool,       # 16KB
            tc.tile_pool(name="k8", bufs=1) as k8_pool,         # 8KB
            tc.tile_pool(name="q8", bufs=1) as q8_pool,         # 16KB
            tc.tile_pool(name="qs8", bufs=1) as qs8_pool,       # 8KB staging
            tc.tile_pool(name="vt", bufs=1) as v_pool,
            tc.tile_pool(name="pt", bufs=1) as p_pool,          # 32KB
            tc.tile_pool(name="outp", bufs=2) as out_pool,
            tc.tile_pool(name="small", bufs=1) as small_pool,
            tc.tile_pool(name="ps", bufs=6, space="PSUM") as ps_pool,
            tc.tile_pool(name="warm", bufs=1, space="PSUM") as warm_pool,
            tc.tile_pool(name="dram", bufs=2, space="DRAM") as dram_pool,
        ):
            bq_sb = small_pool.tile([P, EO], F32, tag="bq")
            bv_sb = small_pool.tile([P, D], BF16, tag="bv")
            ones_sb = small_pool.tile([P, 1], BF16, tag="ones")
            zs_sb = small_pool.tile([P, QT], F32, tag="zs")
            nc.vector.memset(ones_sb[:], 1.0)

            # keep the PE busy (HAM warm) while the first x/W DMAs land
            warm_ps = warm_pool.tile([1, 8], F32, tag="warm")
            for _ in range(120):
                nc.tensor.matmul(
                    warm_ps[:, 0:1], ones_sb[:], ones_sb[:],
                    start=True, stop=True,
                )

            xts = xts_pool.tile([P, DO, XW], BF16, tag="xts")
            wk_sb = wk_pool.tile([P, EO, DO, P], BF16, tag="wk")
            wq_sb = wq_pool.tile([P, EO, DO, P], BF16, tag="wq")
            wve = wve_pool.tile([P, DO, D], BF16, tag="wve")
            k8 = k8_pool.tile([P, EO, KH], FP8, tag="k8")
            q8 = q8_pool.tile([P, EO, S], FP8, tag="q8")
            v_sb = v_pool.tile([P, KO, VW], BF16, tag="vt")
            p_sb = p_pool.tile([P, KO, S], BF16, tag="pt")

            # ---- DMA issue: x + bounce traffic on sync queue, weights on
            # ---- the scalar queue (second hardware DGE) ------------------
            for sq in range(XW // 512):
                for do in range(DO):
                    nc.sync.dma_start(
                        xts[:, do, sq * 512 : sq * 512 + 512],
                        xTr[:, do, sq * 512 : sq * 512 + 512],
                    )
            nc.scalar.dma_start(bq_sb[:], bqT[:, :])
            nc.scalar.dma_start(bv_sb[:], bv[:, :])
            for eo in range(EO):
                nc.scalar.dma_start(
                    wq_sb[:, eo], Wq[eo].unsqueeze(0).rearrange(
                        "o p (do e) -> (o p) do e", do=DO
                    ),
                )
            for eo in range(EO):
                nc.scalar.dma_start(
                    wk_sb[:, eo], Wk[eo].unsqueeze(0).rearrange(
                        "o p (do e) -> (o p) do e", do=DO
                    ),
                )
            for do in range(DO):
                nc.scalar.dma_start(wve[:, do, :], Wvr[:, do, :])
            # ones column of V for the Z ride-along
            nc.vector.memset(v_sb[:, :, D : D + 1], 1.0)

            # ---- Q projection (own query half), bias via ACT Identity ----
            if USE_CC:
                q_stage = qs8_pool.tile([P, EO, QH], FP8, tag="qs8")
                for eo in range(EO):
                    for qc in range(QH // 512):
                        ps0 = ps_pool.tile([P, 512], F32, tag="ps")
                        for do in range(DO):
                            nc.tensor.matmul(
                                ps0[:], wq_sb[:, eo, do],
                                xts[:, do, qc * 512 : qc * 512 + 512],
                                start=(do == 0), stop=(do == DO - 1),
                            )
                        nc.scalar.activation(
                            q_stage[:, eo, qc * 512 : qc * 512 + 512], ps0[:],
                            ACTF.Identity, bias=bq_sb[:, eo : eo + 1],
                        )
                # pair-AllGather: out = [rank0 half, rank1 half] = global order
                cc_in = dram_pool.tile([P, EO, QH], FP8, tag="ccin")
                cc_out = dram_pool.tile([2, P, EO, QH], FP8, tag="ccout")
                nc.sync.dma_start(cc_in[:], q_stage[:])
                nc.gpsimd.collective_compute(
                    "AllGather",
                    mybir.AluOpType.bypass,
                    replica_groups=[[0, 1], [2, 3], [4, 5], [6, 7]],
                    ins=[cc_in[:].opt()],
                    outs=[cc_out[:].opt()],
                )
                for r in range(2):
                    for eo in range(EO):
                        nc.sync.dma_start(
                            q8[:, eo, r * QH : (r + 1) * QH],
                            cc_out[r, :, eo, :],
                        )
            else:
                for eo in range(EO):
                    for qc in range(QC):
                        ps0 = ps_pool.tile([P, 512], F32, tag="ps")
                        for do in range(DO):
                            nc.tensor.matmul(
                                ps0[:], wq_sb[:, eo, do],
                                xts[:, do, qc * 512 : qc * 512 + 512],
                                start=(do == 0), stop=(do == DO - 1),
                            )
                        nc.scalar.activation(
                            q8[:, eo, qc * 512 : qc * 512 + 512], ps0[:],
                            ACTF.Identity, bias=bq_sb[:, eo : eo + 1],
                        )

            # ---- K projection (key half, no bias: cancels in softmax) ----
            for eo in range(EO):
                for kc in range(KC):
                    ps0 = ps_pool.tile([P, 512], F32, tag="ps")
                    for do in range(DO):
                        nc.tensor.matmul(
                            ps0[:], wk_sb[:, eo, do],
                            xts[:, do, kc * 512 : kc * 512 + 512],
                            start=(do == 0), stop=(do == DO - 1),
                        )
                    nc.scalar.activation(
                        k8[:, eo, kc * 512 : kc * 512 + 512], ps0[:], ACTF.Copy,
                    )

            # ---- V projection (key half) ---------------------------------
            for kt in range(KO):
                ps0 = ps_pool.tile([P, 512], F32, tag="ps")
                ps1 = ps_pool.tile([P, 512], F32, tag="ps")
                for do in range(DO):
                    xkt = xts[:, do, kt * P : (kt + 1) * P]
                    nc.tensor.matmul(
                        ps0[:], xkt, wve[:, do, 0:512],
                        start=(do == 0), stop=(do == DO - 1),
                    )
                    nc.tensor.matmul(
                        ps1[:], xkt, wve[:, do, 512:1024],
                        start=(do == 0), stop=(do == DO - 1),
                    )
                nc.vector.tensor_tensor(
                    v_sb[:, kt, 0:512], ps0[:], bv_sb[:, 0:512],
                    mybir.AluOpType.add,
                )
                nc.vector.tensor_tensor(
                    v_sb[:, kt, 512:1024], ps1[:], bv_sb[:, 512:1024],
                    mybir.AluOpType.add,
                )

            # ---- scores^T: exp(K.T @ Q / sqrt(D)), fp8 DoubleRow ---------
            for qc in range(QC):
                for kt in range(KO):
                    ps0 = ps_pool.tile([P, 512], F32, tag="ps")
                    for ep in range(EP):
                        nc.tensor.matmul(
                            ps0[:],
                            k8[:, 2 * ep : 2 * ep + 2, kt * P : (kt + 1) * P],
                            q8[:, 2 * ep : 2 * ep + 2, qc * 512 : qc * 512 + 512],
                            start=(ep == 0), stop=(ep == EP - 1),
                            perf_mode=DR,
                        )
                    nc.scalar.activation(
                        p_sb[:, kt, qc * 512 : qc * 512 + 512], ps0[:],
                        ACTF.Exp, scale=float(SCALE),
                    )

            # ---- attention numerator N = P^T.T @ [V | 1] -----------------
            # chunks 384/384/257: every ldweights hides behind the moving
            # phase of the previous matmul; Z is column 1024.
            for qt in range(QT):
                av0 = ps_pool.tile([P, 512], F32, tag="ps")
                av1 = ps_pool.tile([P, 512], F32, tag="ps")
                av2 = ps_pool.tile([P, 512], F32, tag="ps")
                for ko in range(KO):
                    lhs = p_sb[:, ko, qt * P : (qt + 1) * P]
                    nc.tensor.matmul(
                        av0[:, 0:384], lhs, v_sb[:, ko, 0:384],
                        start=(ko == 0), stop=(ko == KO - 1),
                    )
                    nc.tensor.matmul(
                        av1[:, 0:384], lhs, v_sb[:, ko, 384:768],
                        start=(ko == 0), stop=(ko == KO - 1),
                    )
                    nc.tensor.matmul(
                        av2[:, 0:257], lhs, v_sb[:, ko, 768 : 768 + 257],
                        start=(ko == 0), stop=(ko == KO - 1),
                    )
                o0 = out_pool.tile([P, 384], BF16, tag="o0")
                o1 = out_pool.tile([P, 384], BF16, tag="o1")
                o2 = out_pool.tile([P, 256], BF16, tag="o2")
                nc.scalar.activation(o0[:], av0[:, 0:384], ACTF.Copy)
                nc.scalar.activation(o1[:], av1[:, 0:384], ACTF.Copy)
                nc.scalar.activation(o2[:], av2[:, 0:256], ACTF.Copy)
                nc.scalar.activation(
                    zs_sb[:, qt : qt + 1], av2[:, 256:257], ACTF.Copy
                )
                r0 = qt * P
                nc.sync.dma_start(y[r0 : r0 + P, 0:384], o0[:])
                nc.sync.dma_start(y[r0 : r0 + P, 384:768], o1[:])
                nc.sync.dma_start(y[r0 : r0 + P, 768:1024], o2[:])
            nc.sync.dma_start(z[:, :], zs_sb[:])

    nc.finalize()
    return nc


_NC_CACHE = None


def make_in_maps(x, Wk, bk, Wq, bq, Wv, bv):
    import ml_dtypes

    bf16 = ml_dtypes.bfloat16
    x = np.asarray(x, dtype=np.float32)

    def _wre(W):
        # [D, D] -> [EO, P(part), DO*P] so each e-tile slice is one
        # fully contiguous per-partition DMA
        W = np.asarray(W, np.float32).reshape(DO, P, EO, P)
        return np.ascontiguousarray(
            W.transpose(2, 1, 0, 3).reshape(EO, P, DO * P).astype(bf16)
        )

    Wk8 = _wre(Wk)
    Wq8 = _wre(Wq)
    Wv8 = np.ascontiguousarray(np.asarray(Wv, np.float32).astype(bf16))
    bqT = np.ascontiguousarray(np.asarray(bq, np.float32).reshape(EO, P).T)
    bv2 = np.ascontiguousarray(
        np.broadcast_to(
            np.asarray(bv, np.float32).reshape(1, D), (P, D)
        ).astype(bf16)
    )

    in_maps = []
    for c in range(8):
        b, kh = c // 2, c % 2
        xTb = x[b].T.astype(bf16)                          # [D, S]
        if USE_CC:
            # core's keys AND its projected query half are cols [kh*KH,...)
            xTb = np.ascontiguousarray(xTb[:, kh * KH : (kh + 1) * KH])
        elif kh == 1:
            # swap the s-halves so the key half is always cols [0, KH)
            xTb = np.ascontiguousarray(
                np.concatenate([xTb[:, KH:], xTb[:, :KH]], axis=1)
            )
        else:
            xTb = np.ascontiguousarray(xTb)
        in_maps.append(
            {
                "xT": xTb,
                "Wk": Wk8, "Wq": Wq8, "Wv": Wv8,
                "bqT": bqT, "bv": bv2,
            }
        )
    return in_maps


def gather_out(results):
    out = np.empty((B, S, D), dtype=np.float32)
    for b in range(B):
        r0, r1 = results[2 * b], results[2 * b + 1]
        n0 = np.asarray(r0["y"], dtype=np.float32)          # [S, D]
        n1 = np.asarray(r1["y"], dtype=np.float32)
        z0 = np.asarray(r0["z"], dtype=np.float32).T.reshape(S)
        z1 = np.asarray(r1["z"], dtype=np.float32).T.reshape(S)
        if not USE_CC:
            # core 1 saw queries in swapped-half order; map back to global
            n1 = np.concatenate([n1[KH:], n1[:KH]], axis=0)
            z1 = np.concatenate([z1[KH:], z1[:KH]], axis=0)
        out[b] = (n0 + n1) / (z0 + z1)[:, None]
    return out


def kernel(x, Wk, bk, Wq, bq, Wv, bv):
    global _NC_CACHE
    if _NC_CACHE is None:
        _NC_CACHE = build_nc()
    in_maps = make_in_maps(x, Wk, bk, Wq, bq, Wv, bv)
    res = run_bass_kernel_spmd(_NC_CACHE, in_maps, list(range(8)))
    return gather_out(res.results)


# revision 19
# speedup vs baseline: 1.5607x; 1.0008x over previous
"""Trainium2 Bass kernel for nn_Attention_Layer (B=4, S=2048, D=1024, fp32).

Sharding: 8 cores = 4 batches x 2 KEY-halves (flash-attention style).
Each core computes K/V projections for its 1024-key half, Q for a
1024-query half, AllGathers Q across the pair (the gather's rank-order
concat IS the global query order, so every downstream access stays
SPMD-uniform), computes exp-scores against its key half and ships the
UNNORMALIZED attention output N = exp(S)@V plus the per-query partial
denominator Z. The host combines halves: y = (N0 + N1) / (Z0 + Z1).

Compute dtypes: projections run the PE in bf16 (x and W shipped bf16);
the scores matmul runs in fp8-e4m3 with MatmulPerfMode.DoubleRow (two
128-deep k-tiles contracted per instruction, 2x bf16 rate); exp on ACT
in fp32; attn-weights @ V in bf16. The K-projection bias is dropped:
score terms that depend only on the query row cancel in softmax.

Z rides along as column 1024 (all-ones) of the V tile: the attn @ V
matmuls per (qt, ko) are 384+384+257 wide, so every weight (re)load
hides behind a longer moving phase and Z needs no extra 1-wide matmuls.

DMA uses both hardware DGE queues: weights stream on the scalar queue,
x / bounce / outputs on the sync queue, so the first weight tile isn't
stuck behind the x stream. A short burst of dummy matmuls at kernel
start keeps the PE's HAM clock gate warm through the initial DMA wait.
"""

import numpy as np

import concourse.mybir as mybir
import concourse.tile as tile
from concourse import bacc
from concourse.bass_utils import run_bass_kernel_spmd

B, S, D = 4, 2048, 1024
P = 128
KH = S // 2              # keys per core
QH = S // 2              # queries projected per core (CC mode)
EO = D // P              # 8 e-tiles (feature dim outer)
DO = D // P              # 8 d-tiles (contraction outer)
KO = KH // P             # 8 key 128-tiles per core
QC = S // 512            # 4 query 512-chunks
KC = KH // 512           # 2 key 512-chunks
EP = EO // 2             # 4 eo-pairs for DoubleRow
QT = S // P              # 16 query 128-tiles
VW = D + 1               # V free width incl. the ones column for Z
SCALE = 1.0 / np.sqrt(D)

USE_CC = True            # AllGather the Q projection across core pairs

F32 = mybir.dt.float32
BF16 = mybir.dt.bfloat16
FP8 = mybir.dt.float8e4
DR = mybir.MatmulPerfMode.DoubleRow
ACTF = mybir.ActivationFunctionType


def build_nc():
    nc = bacc.Bacc("TRN2", target_bir_lowering=False)

    XW = S if not USE_CC else KH     # x columns actually needed on-core
    xT = nc.dram_tensor("xT", [D, XW], BF16, kind="ExternalInput")
    Wk = nc.dram_tensor("Wk", [EO, P, DO * P], BF16, kind="ExternalInput")
    Wq = nc.dram_tensor("Wq", [EO, P, DO * P], BF16, kind="ExternalInput")
    Wv = nc.dram_tensor("Wv", [D, D], BF16, kind="ExternalInput")
    bqT = nc.dram_tensor("bqT", [P, EO], F32, kind="ExternalInput")
    bv = nc.dram_tensor("bv", [P, D], BF16, kind="ExternalInput")
    onesd = nc.dram_tensor("ones", [P, 1], BF16, kind="ExternalInput")
    y = nc.dram_tensor("y", [S, D], BF16, kind="ExternalOutput")
    z = nc.dram_tensor("z", [P, QT], F32, kind="ExternalOutput")

    xTr = xT.ap().rearrange("(do p) s -> p do s", p=P)
    Wvr = Wv.ap().rearrange("(do p) e -> p do e", p=P)

    with tile.TileContext(nc) as tc:
        with (
            tc.tile_pool(name="xts", bufs=1) as xts_pool,
            tc.tile_pool(name="wk", bufs=1) as wk_pool,         # 16KB
            tc.tile_pool(name="wq", bufs=1) as wq_pool,         # 16KB
            tc.tile_pool(name="wve", bufs=1) as wve_pool,       # 16KB
            tc.tile_pool(name="k8", bufs=1) as k8_pool,         # 8KB
            tc.tile_pool(name="q8", bufs=1) as q8_pool,         # 16KB
            tc.tile_pool(name="qs8", bufs=1) as qs8_pool,       # 8KB staging
            tc.tile_pool(name="vt", bufs=1) as v_pool,
            tc.tile_pool(name="pt", bufs=1) as p_pool,          # 32KB
            tc.tile_pool(name="outp", bufs=3) as out_pool,
            tc.tile_pool(name="small", bufs=1) as small_pool,
            tc.tile_pool(name="ps", bufs=6, space="PSUM") as ps_pool,
            tc.tile_pool(name="warm", bufs=1, space="PSUM") as warm_pool,
            tc.tile_pool(name="dram", bufs=2, space="DRAM") as dram_pool,
        ):
            bq_sb = small_pool.tile([P, EO], F32, tag="bq")
            bv_sb = small_pool.tile([P, D], BF16, tag="bv")
            ones_sb = small_pool.tile([P, 1], BF16, tag="ones")
            zs_sb = small_pool.tile([P, QT], F32, tag="zs")

            # keep the PE busy (HAM warm) while the first x/W DMAs land.
            # ones comes via a tiny leading DMA, not a vector memset, so the
            # warmup doesn't wait for the vector-engine preamble.
            nc.sync.dma_start(ones_sb[:], onesd[:, :])
            warm_ps = warm_pool.tile([1, 8], F32, tag="warm")
            for _ in range(120):
                nc.tensor.matmul(
                    warm_ps[:, 0:1], ones_sb[:], ones_sb[:],
                    start=True, stop=True,
                )

            xts = xts_pool.tile([P, DO, XW], BF16, tag="xts")
            wk_sb = wk_pool.tile([P, EO, DO, P], BF16, tag="wk")
            wq_sb = wq_pool.tile([P, EO, DO, P], BF16, tag="wq")
            wve = wve_pool.tile([P, DO, D], BF16, tag="wve")
            k8 = k8_pool.tile([P, EO, KH], FP8, tag="k8")
            q8 = q8_pool.tile([P, EO, S], FP8, tag="q8")
            v_sb = v_pool.tile([P, KO, VW], BF16, tag="vt")
            p_sb = p_pool.tile([P, KO, S], BF16, tag="pt")

            # ---- DMA issue: x split across both hardware DGE queues
            # ---- (2KB contiguous per-partition lines), weights on the
            # ---- scalar queue, outputs later on the scalar queue ---------
            for do in range(DO // 2):
                nc.sync.dma_start(xts[:, do, :], xTr[:, do, 0:XW])
            nc.scalar.dma_start(bq_sb[:], bqT[:, :])
            nc.scalar.dma_start(bv_sb[:], bv[:, :])
            for do in range(DO // 2, DO):
                nc.scalar.dma_start(xts[:, do, :], xTr[:, do, 0:XW])
            for eo in range(EO):
                nc.scalar.dma_start(
                    wq_sb[:, eo], Wq[eo].unsqueeze(0).rearrange(
                        "o p (do e) -> (o p) do e", do=DO
                    ),
                )
            for eo in range(EO):
                nc.scalar.dma_start(
                    wk_sb[:, eo], Wk[eo].unsqueeze(0).rearrange(
                        "o p (do e) -> (o p) do e", do=DO
                    ),
                )
            for do in range(DO):
                nc.scalar.dma_start(wve[:, do, :], Wvr[:, do, :])
            # ones column of V for the Z ride-along
            nc.vector.memset(v_sb[:, :, D : D + 1], 1.0)

            # ---- Q projection (own query half), bias via ACT Identity ----
            if USE_CC:
                q_stage = qs8_pool.tile([P, EO, QH], FP8, tag="qs8")
                for eo in range(EO):
                    for qc in range(QH // 512):
                        ps0 = ps_pool.tile([P, 512], F32, tag="ps")
                        for do in range(DO):
                            nc.tensor.matmul(
                                ps0[:], wq_sb[:, eo, do],
                                xts[:, do, qc * 512 : qc * 512 + 512],
                                start=(do == 0), stop=(do == DO - 1),
                            )
                        nc.scalar.activation(
                            q_stage[:, eo, qc * 512 : qc * 512 + 512], ps0[:],
                            ACTF.Identity, bias=bq_sb[:, eo : eo + 1],
                        )
                # pair-AllGather: out = [rank0 half, rank1 half] = global order
                cc_in = dram_pool.tile([P, EO, QH], FP8, tag="ccin")
                cc_out = dram_pool.tile([2, P, EO, QH], FP8, tag="ccout")
                nc.scalar.dma_start(cc_in[:], q_stage[:])
                nc.gpsimd.collective_compute(
                    "AllGather",
                    mybir.AluOpType.bypass,
                    replica_groups=[[0, 1], [2, 3], [4, 5], [6, 7]],
                    ins=[cc_in[:].opt()],
                    outs=[cc_out[:].opt()],
                )
                for r in range(2):
                    for eo in range(EO):
                        nc.sync.dma_start(
                            q8[:, eo, r * QH : (r + 1) * QH],
                            cc_out[r, :, eo, :],
                        )
            else:
                for eo in range(EO):
                    for qc in range(QC):
                        ps0 = ps_pool.tile([P, 512], F32, tag="ps")
                        for do in range(DO):
                            nc.tensor.matmul(
                                ps0[:], wq_sb[:, eo, do],
                                xts[:, do, qc * 512 : qc * 512 + 512],
                                start=(do == 0), stop=(do == DO - 1),
                            )
                        nc.scalar.activation(
                            q8[:, eo, qc * 512 : qc * 512 + 512], ps0[:],
                            ACTF.Identity, bias=bq_sb[:, eo : eo + 1],
                        )

            # ---- K projection (key half, no bias: cancels in softmax) ----
            for eo in range(EO):
                for kc in range(KC):
                    ps0 = ps_pool.tile([P, 512], F32, tag="ps")
                    for do in range(DO):
                        nc.tensor.matmul(
                            ps0[:], wk_sb[:, eo, do],
                            xts[:, do, kc * 512 : kc * 512 + 512],
                            start=(do == 0), stop=(do == DO - 1),
                        )
                    nc.scalar.activation(
                        k8[:, eo, kc * 512 : kc * 512 + 512], ps0[:], ACTF.Copy,
                    )

            # ---- V projection (key half) ---------------------------------
            for kt in range(KO):
                ps0 = ps_pool.tile([P, 512], F32, tag="ps")
                ps1 = ps_pool.tile([P, 512], F32, tag="ps")
                for do in range(DO):
                    xkt = xts[:, do, kt * P : (kt + 1) * P]
                    nc.tensor.matmul(
                        ps0[:], xkt, wve[:, do, 0:512],
                        start=(do == 0), stop=(do == DO - 1),
                    )
                    nc.tensor.matmul(
                        ps1[:], xkt, wve[:, do, 512:1024],
                        start=(do == 0), stop=(do == DO - 1),
                    )
                nc.vector.tensor_tensor(
                    v_sb[:, kt, 0:512], ps0[:], bv_sb[:, 0:512],
                    mybir.AluOpType.add,
                )
                nc.vector.tensor_tensor(
                    v_sb[:, kt, 512:1024], ps1[:], bv_sb[:, 512:1024],
                    mybir.AluOpType.add,
                )

            # ---- scores^T: exp(K.T @ Q / sqrt(D)), fp8 DoubleRow ---------
            for qc in range(QC):
                for kt in range(KO):
                    ps0 = ps_pool.tile([P, 512], F32, tag="ps")
                    for ep in range(EP):
                        nc.tensor.matmul(
                            ps0[:],
                            k8[:, 2 * ep : 2 * ep + 2, kt * P : (kt + 1) * P],
                            q8[:, 2 * ep : 2 * ep + 2, qc * 512 : qc * 512 + 512],
                            start=(ep == 0), stop=(ep == EP - 1),
                            perf_mode=DR,
                        )
                    nc.scalar.activation(
                        p_sb[:, kt, qc * 512 : qc * 512 + 512], ps0[:],
                        ACTF.Exp, scale=float(SCALE),
                    )

            # ---- attention numerator N = P^T.T @ [V | 1] -----------------
            # chunks 384/384/257: every ldweights hides behind the moving
            # phase of the previous matmul; Z is column 1024.
            for qt in range(QT):
                av0 = ps_pool.tile([P, 512], F32, tag="ps")
                av1 = ps_pool.tile([P, 512], F32, tag="ps")
                av2 = ps_pool.tile([P, 512], F32, tag="ps")
                for ko in range(KO):
                    lhs = p_sb[:, ko, qt * P : (qt + 1) * P]
                    nc.tensor.matmul(
                        av0[:, 0:384], lhs, v_sb[:, ko, 0:384],
                        start=(ko == 0), stop=(ko == KO - 1),
                    )
                    nc.tensor.matmul(
                        av1[:, 0:384], lhs, v_sb[:, ko, 384:768],
                        start=(ko == 0), stop=(ko == KO - 1),
                    )
                    nc.tensor.matmul(
                        av2[:, 0:257], lhs, v_sb[:, ko, 768 : 768 + 257],
                        start=(ko == 0), stop=(ko == KO - 1),
                    )
                oy = out_pool.tile([P, D], BF16, tag="oy")
                nc.scalar.activation(oy[:, 0:384], av0[:, 0:384], ACTF.Copy)
                nc.scalar.activation(oy[:, 384:768], av1[:, 0:384], ACTF.Copy)
                nc.scalar.activation(oy[:, 768:1024], av2[:, 0:256], ACTF.Copy)
                nc.scalar.activation(
                    zs_sb[:, qt : qt + 1], av2[:, 256:257], ACTF.Copy
                )
                nc.scalar.dma_start(y[qt * P : (qt + 1) * P, :], oy[:])
            nc.sync.dma_start(z[:, :], zs_sb[:])

    nc.finalize()
    return nc


_NC_CACHE = None


def make_in_maps(x, Wk, bk, Wq, bq, Wv, bv):
    import ml_dtypes

    bf16 = ml_dtypes.bfloat16
    x = np.asarray(x, dtype=np.float32)

    def _wre(W):
        # [D, D] -> [EO, P(part), DO*P] so each e-tile slice is one
        # fully contiguous per-partition DMA
        W = np.asarray(W, np.float32).reshape(DO, P, EO, P)
        return np.ascontiguousarray(
            W.transpose(2, 1, 0, 3).reshape(EO, P, DO * P).astype(bf16)
        )

    Wk8 = _wre(Wk)
    Wq8 = _wre(Wq)
    Wv8 = np.ascontiguousarray(np.asarray(Wv, np.float32).astype(bf16))
    bqT = np.ascontiguousarray(np.asarray(bq, np.float32).reshape(EO, P).T)
    bv2 = np.ascontiguousarray(
        np.broadcast_to(
            np.asarray(bv, np.float32).reshape(1, D), (P, D)
        ).astype(bf16)
    )

    in_maps = []
    for c in range(8):
        b, kh = c // 2, c % 2
        xTb = x[b].T.astype(bf16)                          # [D, S]
        if USE_CC:
            # core's keys AND its projected query half are cols [kh*KH,...)
            xTb = np.ascontiguousarray(xTb[:, kh * KH : (kh + 1) * KH])
        elif kh == 1:
            # swap the s-halves so the key half is always cols [0, KH)
            xTb = np.ascontiguousarray(
                np.concatenate([xTb[:, KH:], xTb[:, :KH]], axis=1)
            )
        else:
            xTb = np.ascontiguousarray(xTb)
        in_maps.append(
            {
                "xT": xTb,
                "Wk": Wk8, "Wq": Wq8, "Wv": Wv8,
                "bqT": bqT, "bv": bv2,
                "ones": np.ones((P, 1), dtype=bf16),
            }
        )
    return in_maps


def gather_out(results):
    out = np.empty((B, S, D), dtype=np.float32)
    for b in range(B):
        r0, r1 = results[2 * b], results[2 * b + 1]
        n0 = np.asarray(r0["y"], dtype=np.float32)          # [S, D]
        n1 = np.asarray(r1["y"], dtype=np.float32)
        z0 = np.asarray(r0["z"], dtype=np.float32).T.reshape(S)
        z1 = np.asarray(r1["z"], dtype=np.float32).T.reshape(S)
        if not USE_CC:
            # core 1 saw queries in swapped-half order; map back to global
            n1 = np.concatenate([n1[KH:], n1[:KH]], axis=0)
            z1 = np.concatenate([z1[KH:], z1[:KH]], axis=0)
        out[b] = (n0 + n1) / (z0 + z1)[:, None]
    return out


def kernel(x, Wk, bk, Wq, bq, Wv, bv):
    global _NC_CACHE
    if _NC_CACHE is None:
        _NC_CACHE = build_nc()
    in_maps = make_in_maps(x, Wk, bk, Wq, bq, Wv, bv)
    res = run_bass_kernel_spmd(_NC_CACHE, in_maps, list(range(8)))
    return gather_out(res.results)
